# revision 13
# baseline (speedup 1.0000x reference)
"""Trainium2 Bass kernel for nn_ARTLayer (gnn_message_passing).

Math (reference):
    j(i,t) = t + (t>=i)                                    # [K, K-1] neighbor index
    alpha  = sigmoid(x@wa [i] + x@wb [j] + pf@wc + b_att)  # [K, K-1]
    msgs   = mean_t alpha * ((x@WobjT + b_obj)[j] + pf@WpairT + b_pair)
    out    = LN(x + msgs); out = LN(out + FFN(out))

Key algebraic rewrite (removes the 34-GFLOP [P,PD]x[PD,H] einsum):
    sum_t a*(pf@WpT)  = (sum_t a*pf) @ WpT               -> U[i,:] @ WpT
    sum_t a*oj[j]     = (A @ x) @ WobjT                  -> Gx[i,:] @ WoT
      with A[i,j] decomposed via lo/hi shifted views of x and a t>=i mask
    sum_t a*(b_obj+b_pair) = s_alpha[i] * bop

Sharding: rows i split across 8 cores (64 each); small tensors replicated;
host concatenates the per-core [64, 512] outputs.

V2 schedule (rebuilt from NTFF trace analysis of the 88us baseline):
  - pf streams on the sync HWDGE queue right behind the packed consts, in
    8 half-chunk DMAs, so the DVE reduce tree starts at ~6us instead of 29us.
  - sa/sb use fused scalar_tensor_tensor with accum_out (one op instead of
    mul+reduce); tree tail levels fused into one tensor_reduce.
  - Gx and the dx correction accumulate *transposed* in PSUM (xlo/dx blocks
    as the matmul stationary, alpha as the moving operand) - no PE
    transposes of gx in the tail. s_alpha accumulated as a column the same
    way.
  - U diagonal extraction streams a [128,4] selection matrix instead of a
    full 128-col transpose; one 4D-AP gather copy assembles U^T.
  - msg/f1 biases are preloaded into PSUM by the DVE (no rank-1 matmuls in
    PE accumulation groups); FFN layer 1 computed directly transposed
    (W1T blocks stationary) so only one transpose set (out1) remains.
  - LN epilogues fused via scalar_tensor_tensor; sqrt table switch forced
    right after the last sigmoid via a dummy sqrt.
  - Tail-only weights ride the ACT HWDGE queue, position-staggered between
    sigmoids so they never contend with the pf stream for HBM.
"""
import numpy as np

import concourse.bass as bass
import concourse.tile as tile
from concourse import bacc, mybir

F32, F16 = mybir.dt.float32, mybir.dt.float16
AX = mybir.AxisListType
OP = mybir.AluOpType
AF = mybir.ActivationFunctionType

DEBUG = False
K, D, H, PD = 512, 512, 512, 128
T = K - 1                      # 511 neighbors per row
NCORES, IPC = 8, 64            # rows per core
NCH = 4                        # t-chunks of 128 (last chunk row 127 is t=511 pad)
IB = 64                        # i-block within a core
EPS = 1e-5


def build_program() -> bacc.Bacc:
    nc = bacc.Bacc("TRN2", target_bir_lowering=False, debug=False)

    def inp(name, shape, dt):
        return nc.dram_tensor(name, shape, dt, kind="ExternalInput").ap()

    pf = inp("pf", [NCH, 2, 128, 32, PD], F16)  # [chunk, i-half, t, i, pd]
    # single-DMA packed critical constants (128 contiguous rows each):
    # b32: [cmat 0:512 | poison 512:640 | mask_lt 640:896 | b_att 896:904
    #       | b1pT 904:908]
    # b16: [wa_t 0:512 | wb_t 512:1024 | mask_ge 1024:1280 | ones 1280:1288
    #       | xi16 1288:1800 | sel 1800:1816 | id16 1816:1944]
    b32 = inp("b32", [128, 262], F32)
    b16 = inp("b16", [128, 1300], F16)
    xlo_ch = inp("xlo_ch", [128, NCH, D], F16)  # x rows chunked [t%128, t//128]
    dxf = inp("dxf", [K, D], F16)               # x[t+1] - x[t], host computed
    xi = inp("xi", [IPC, D], F32)               # this core's rows of x
    bias4 = inp("bias4", [4, H], F32)           # [ln_g; ln_b; lnb+b2; bop/511]
    WpT = inp("WpT", [PD, H], F16)              # W_pair.T / colscale / 511
    WoT = inp("WoT", [D, H], F16)               # W_obj.T / 511
    W1T = inp("W1T", [H, H], F16)               # W1.T * ln_g
    W2T = inp("W2T", [H, H], F16)

    out_d = nc.dram_tensor("out", [IPC, H], F32, kind="ExternalOutput").ap()
    if DEBUG:
        dbg_alpha = nc.dram_tensor(
            "dbg_alpha", [128, NCH, IPC], F16, kind="ExternalOutput").ap()
        dbg_h = nc.dram_tensor("dbg_h", [IPC, H], F32, kind="ExternalOutput").ap()
        dbg_cen = nc.dram_tensor("dbg_cen", [IPC, H], F16, kind="ExternalOutput").ap()
        dbg_f1 = nc.dram_tensor(
            "dbg_f1", [128, 4, IPC], F16, kind="ExternalOutput").ap()
        dbg_gxT = nc.dram_tensor(
            "dbg_gxT", [128, 4, IPC], F16, kind="ExternalOutput").ap()
        dbg_u = nc.dram_tensor("dbg_u", [128, IPC], F16, kind="ExternalOutput").ap()

    with tile.TileContext(nc) as tc:
        with (
            tc.tile_pool(name="const", bufs=1) as cpool,
            tc.tile_pool(name="pfp", bufs=4) as pfp,
            tc.tile_pool(name="scrp", bufs=2) as scrp,
            tc.tile_pool(name="smallp", bufs=4) as smallp,
            tc.tile_pool(name="postp", bufs=3) as postp,
            tc.tile_pool(name="pss", bufs=2, space="PSUM") as pss,
            tc.tile_pool(name="psu", bufs=4, space="PSUM") as psu,
            tc.tile_pool(name="psg", bufs=1, space="PSUM") as psg,
            tc.tile_pool(name="psm", bufs=1, space="PSUM") as psm,
        ):
            def dma(out, in_):
                nc.sync.dma_start(out=out, in_=in_)

            def dma_w(out, in_):
                nc.scalar.dma_start(out=out, in_=in_)

            # ---- critical consts, then the pf stream, on the sync queue ----
            hp = tc.high_priority()
            hp.__enter__()
            b16_sb = cpool.tile([128, 1300], F16)
            dma(b16_sb, b16)
            xlo = cpool.tile([128, NCH, D], F16)
            dma(xlo, xlo_ch)
            b32_sb = cpool.tile([128, 262], F32)
            dma(b32_sb, b32)
            hp.__exit__(None, None, None)
            tc.no_sync_barrier()   # keep pf DMAs behind the critical consts

            pf_t = []
            for c in range(NCH):
                t_ = pfp.tile([128, IB, PD], F16, tag="pf_t", name=f"pf{c}")
                pf_t.append(t_)
                for h in range(2):
                    dma(t_[:, 32 * h:32 * h + 32, :], pf[c, h])

            sh1_sb = b32_sb[:, 0:128]
            sh2_sb = b32_sb[:, 128:256]
            poison_col = b32_sb[:, 256:257]
            b_att_1 = b32_sb[0:1, 257:258]
            b1pT = b32_sb[:, 258:262]
            wb_b = b16_sb[:, 0:512]
            mge_sb = b16_sb[:, 512:768].rearrange("p (c i) -> p c i", c=NCH)
            ones16_col = b16_sb[:, 768:769]
            ones16_row = b16_sb[0:1, 768:896]
            wa4 = b16_sb[:, 896:900]
            xiT_sb = b16_sb[:, 900:1156].rearrange("p (c i) -> p c i", c=NCH)
            sel_sb = b16_sb[:, 1156:1172].rearrange("p (j s) -> p j s", j=4)
            id16_sb = b16_sb[:, 1172:1300]

            eps_col = cpool.tile([IPC, 1], F32)
            nc.vector.memset(eps_col, EPS)
            zero_col = cpool.tile([128, 1], F32)
            nc.vector.memset(zero_col, 0.0)

            # ---- sb (all rows) fused; sa broadcast via a rank-1 matmul ----
            # sbj[t,i] = sa[i] + b_att + sb_lo[t] + mge[t,i]*(sb_hi-sb_lo)[t]
            # (poison -1e9 folded into sb_lo at the t=511 pad slot)
            sb_cols = smallp.tile([128, NCH], F32)
            for c in range(NCH):
                scr_sb = smallp.tile([128, D], F16, tag="scr_sb")
                nc.vector.scalar_tensor_tensor(
                    scr_sb, xlo[:, c, :], 0.0, wb_b, OP.bypass, OP.mult,
                    accum_out=sb_cols[:, c:c + 1])

            # sb_hi[p, c] = sb[c*128+p+1] via shift matmuls; slot 511 stays 0
            sbhi_ps = pss.tile([128, NCH], F32, tag="ps_small")
            nc.tensor.matmul(sbhi_ps, sh1_sb, sb_cols, start=True, stop=False)
            nc.tensor.matmul(sbhi_ps[:, 0:NCH - 1], sh2_sb, sb_cols[:, 1:NCH],
                             start=False, stop=True)
            dneg = smallp.tile([128, NCH], F32)
            nc.vector.tensor_tensor(dneg, sbhi_ps, sb_cols, OP.subtract)
            nc.vector.tensor_add(sb_cols[:, 3:4], sb_cols[:, 3:4], poison_col)

            # sa row: wa-block columns stationary over xT blocks, then a
            # rank-1 ones matmul broadcasts it across all t partitions
            sarow_ps = pss.tile([1, IPC], F32, tag="ps_small")
            for db in range(4):
                nc.tensor.matmul(sarow_ps, wa4[:, db:db + 1],
                                 xiT_sb[:, db, :],
                                 start=(db == 0), stop=(db == 3))
            sarow16 = smallp.tile([1, IPC], F16)
            nc.vector.tensor_scalar(sarow16, sarow_ps, b_att_1, None,
                                    OP.add, OP.bypass)
            x_ps = pss.tile([128, IPC], F32, tag="ps_small")
            nc.tensor.matmul(x_ps, ones16_row, sarow16, start=True, stop=True)

            sbj = cpool.tile([128, NCH, IPC], F32)
            for c in range(NCH):
                x3c = smallp.tile([128, IPC], F32, tag="x3c")
                nc.vector.tensor_scalar(x3c, x_ps, sb_cols[:, c:c + 1], None,
                                        OP.add, OP.bypass)
                nc.vector.scalar_tensor_tensor(
                    sbj[:, c, :], mge_sb[:, c, :], dneg[:, c:c + 1], x3c,
                    OP.mult, OP.add)

            # ---- main edge pass ----
            alpha_full = cpool.tile([128, NCH, IPC], F16)   # raw sigmoid out
            age_full = cpool.tile([128, NCH, IPC], F16)     # masked (t>=i) alpha
            # gxT[d, i] (4 d-blocks) + s_alpha column, one PSUM bank
            gs_ps = psg.tile([128, 5, IPC], F32)
            nc.vector.memset(gs_ps, 0.0)
            u_ps = [psu.tile([128, 512], F32, tag="flex", name=f"u_ps{b}")
                    for b in range(4)]
            for b in range(4):
                nc.vector.memset(u_ps[b], 0.0)

            def late_dma(dst, src_dram, when):
                # fake WAW: a 1-elem gpsimd copy into dst that reads `when`
                # pins the weight DMA behind that chunk's sigmoid, so the
                # transfer never races the pf stream for HBM.
                nc.gpsimd.tensor_copy(dst[0:1, 0:1], when)
                dma_w(dst, src_dram)

            for c in range(NCH):
                scr = scrp.tile([128, IB, 64], F16, tag="scr")
                # halved L1/L2 so the tree starts as soon as each half lands
                for h in range(2):
                    ih = slice(32 * h, 32 * h + 32)
                    nc.vector.tensor_add(
                        scr[:, ih, :], pf_t[c][:, ih, 0:64],
                        pf_t[c][:, ih, 64:128])
                    nc.vector.tensor_add(
                        scr[:, ih, 0:32], scr[:, ih, 0:32], scr[:, ih, 32:64])
                deep = nc.vector if c == NCH - 1 else nc.gpsimd
                deep.tensor_add(
                    scr[:, :, 0:16], scr[:, :, 0:16], scr[:, :, 16:32])
                deep.tensor_add(
                    scr[:, :, 0:8], scr[:, :, 0:8], scr[:, :, 8:16])
                sc_t = smallp.tile([128, IB], F32, tag="sc_t")
                nc.vector.tensor_reduce(sc_t, scr[:, :, 0:8], axis=AX.X,
                                        op=OP.add)
                aarg = smallp.tile([128, IB], F32, tag="aarg")
                nc.vector.tensor_add(aarg, sc_t, sbj[:, c, :])
                nc.scalar.activation(alpha_full[:, c, :], aarg, AF.Sigmoid,
                                     bias=zero_col)
                nc.gpsimd.tensor_mul(age_full[:, c, :], alpha_full[:, c, :],
                                     mge_sb[:, c, :])
                if c == 0:
                    dx = cpool.tile([128, NCH, D], F16)
                    late_dma(dx.rearrange("p c d -> p (c d)"),
                             dxf.rearrange("(c p) d -> p c d", p=128),
                             alpha_full[0:1, 0, 0:1])
                elif c == 1:
                    WoT_sb = cpool.tile([128, NCH, H], F16)
                    late_dma(WoT_sb.rearrange("p c h -> p (c h)"),
                             WoT.rearrange("(c p) h -> p c h", p=128),
                             alpha_full[0:1, 1, 0:1])
                    WpT_sb = cpool.tile([128, H], F16)
                    late_dma(WpT_sb, WpT, alpha_full[0:1, 1, 0:1])
                elif c == 2:
                    xi_sb = cpool.tile([IPC, D], F32)
                    nc.vector.tensor_copy(xi_sb[0:1, 0:1],
                                          alpha_full[0:1, 2, 0:1])
                    dma_w(xi_sb, xi)
                    bias_sb = cpool.tile([IPC, 4, H], F32)
                    nc.vector.tensor_copy(bias_sb[0:1, 0, 0:1],
                                          alpha_full[0:1, 2, 0:1])
                    dma_w(bias_sb, bias4[None, :, :].to_broadcast([IPC, 4, H]))
                # U quads: lhsT = 4 alpha columns, rhs = 4 pf blocks; the
                # wanted rows sit on the block diagonal
                for q in range(IB // 4):
                    b, sp = divmod(q, 4)
                    nc.tensor.matmul(
                        u_ps[b][32 * sp:32 * sp + 4, :],
                        alpha_full[:, c, 4 * q:4 * q + 4],
                        pf_t[c][:, 4 * q:4 * q + 4, :],
                        start=(c == 0), stop=(c == NCH - 1),
                        tile_position=(0, 32 * sp))
                # gxT accumulates transposed: xlo block stationary, alpha
                # moving.  s_alpha as a column via the ones vector.
                for db in range(4):
                    nc.tensor.matmul(gs_ps[:, db, :],
                                     xlo[:, c, 128 * db:128 * db + 128],
                                     alpha_full[:, c, :],
                                     start=False, stop=False,
                                     skip_group_check=True)
                nc.tensor.matmul(gs_ps[0:IPC, 4, 0:1], alpha_full[:, c, :],
                                 ones16_col, start=False,
                                 stop=(c == NCH - 1), skip_group_check=True)
                # dx correction for the PREVIOUS chunk (dx lands mid-loop)
                if c >= 1:
                    for db in range(4):
                        nc.tensor.matmul(gs_ps[:, db, :],
                                         dx[:, c - 1, 128 * db:128 * db + 128],
                                         age_full[:, c - 1, :],
                                         start=False, stop=False,
                                         skip_group_check=True)
                tc.no_sync_barrier()

            gb_sb = bias_sb[:, 0, :]
            bb_sb = bias_sb[:, 1, :]
            lb2_sb = bias_sb[:, 2, :]
            bop_sb = bias_sb[:, 3, :]

            # scheduler fence: keep every tail instruction after the loop in
            # each engine stream (strict-FIFO engines head-of-line block)
            tc.no_sync_barrier()

            for db in range(4):
                nc.tensor.matmul(gs_ps[:, db, :],
                                 dx[:, 3, 128 * db:128 * db + 128],
                                 age_full[:, 3, :],
                                 start=False, stop=True,
                                 skip_group_check=True)

            # ---- messages = U@WpT + GxT^T@WoT + s_alpha x bop ----
            msg_ps = psm.tile([IPC, H], F32)

            # bop rank-1 preloaded into PSUM by the DVE (no PE rank-1 matmul)
            s_col = smallp.tile([IPC, 1], F32)
            nc.vector.tensor_copy(s_col, gs_ps[0:IPC, 4, 0:1])
            gxT_sb = postp.tile([128, 4, IPC], F16)
            nc.vector.tensor_copy(gxT_sb, gs_ps[:, 0:4, :])
            nc.vector.tensor_scalar(msg_ps, bop_sb, s_col, None,
                                    OP.mult, OP.bypass)

            for db in range(4):
                nc.tensor.matmul(msg_ps, gxT_sb[:, db, :], WoT_sb[:, db, :],
                                 start=False, stop=False, skip_group_check=True)

            # U extraction: copy each PSUM bank to SBUF (2 on DVE, 2 on ACT),
            # then a [128,4] selection matmul per (bank, col-block) pulls the
            # diagonal quads out transposed; one 4D gather copy assembles U^T.
            slots_ps = pss.tile([128, IB], F32, tag="ps_small")
            slots3 = slots_ps.rearrange("p (g s) -> p g s", g=16)
            ucp = []
            for b in range(4):
                u_cp = postp.tile([128, 512], F16, tag="u_cp", name=f"ucp{b}")
                ucp.append(u_cp)
                if b % 2 == 0:
                    nc.vector.tensor_copy(u_cp, u_ps[b])
                else:
                    nc.scalar.copy(u_cp, u_ps[b])
            for b in range(4):
                for j in range(4):
                    nc.tensor.matmul(slots3[:, 4 * b + j, :],
                                     ucp[b][:, 128 * j:128 * j + 128],
                                     sel_sb[:, j, :],
                                     start=True, stop=True,
                                     skip_group_check=True)
            # sqrt table switch + late FFN weights ride the ACT queue
            # behind the ucp copies (anchored so they cannot hoist)
            junk = smallp.tile([1, 1], F32, tag="junk")
            nc.scalar.activation(junk, eps_col[0:1, :], AF.Sqrt)
            W1T_sb = cpool.tile([128, NCH, H], F16)
            late_dma(W1T_sb.rearrange("p c h -> p (c h)"),
                     W1T.rearrange("(c p) h -> p c h", p=128),
                     ucp[1][0:1, 0:1])
            W2T_sb = cpool.tile([128, NCH, H], F16)
            late_dma(W2T_sb.rearrange("p c h -> p (c h)"),
                     W2T.rearrange("(c p) h -> p c h", p=128),
                     ucp[1][0:1, 0:1])

            u_sb = postp.tile([128, IB], F16)
            nc.vector.tensor_copy(
                u_sb.rearrange("p (b s j) -> p b s j", b=4, s=4),
                slots_ps.rearrange("p (b j s) -> p b s j", b=4, j=4))
            nc.tensor.matmul(msg_ps, u_sb, WpT_sb, start=False, stop=True,
                             skip_group_check=True)

            # ---- residual + LN1 (bare: ln_g/ln_b folded into W1T'/b1p) ----
            def ln_stats(v):
                stats = smallp.tile([IPC, 6], F32, tag="stats")
                nc.vector.bn_stats(out=stats, in_=v)
                mv = smallp.tile([IPC, 2], F32, tag="mv")
                nc.vector.bn_aggr(out=mv, in_=stats)
                std = smallp.tile([IPC, 1], F32, tag="std")
                nc.scalar.activation(std, mv[:, 1:2], AF.Sqrt, bias=eps_col)
                rstd = smallp.tile([IPC, 1], F32, tag="rstd")
                nc.vector.reciprocal(rstd, std)
                return mv, rstd

            h_sb = postp.tile([IPC, H], F32)
            nc.vector.tensor_add(h_sb, xi_sb, msg_ps)
            mv1, rstd1 = ln_stats(h_sb)
            cen = postp.tile([IPC, H], F16)
            nc.vector.tensor_scalar(cen, h_sb, mv1[:, 0:1], rstd1,
                                    OP.subtract, OP.mult)

            # ---- FFN: f1T computed directly transposed (W1T stationary) ----
            o1T_ps = pss.tile([128, 4, IPC], F16, tag="ps_small")
            for db in range(4):
                nc.tensor.transpose(o1T_ps[:, db, :],
                                    cen[:, 128 * db:128 * db + 128],
                                    id16_sb[0:IPC, 0:IPC])
            o1T_sb = postp.tile([128, 4, IPC], F16)
            nc.vector.tensor_copy(o1T_sb, o1T_ps)

            # true out1*g + (ln_b + b2) rebuilt off the critical path
            o1b = postp.tile([IPC, H], F32)
            nc.vector.scalar_tensor_tensor(
                o1b, cen, 0.0, gb_sb, OP.bypass, OP.mult)
            nc.vector.tensor_add(o1b, o1b, lb2_sb)

            f1T_ps = psu.tile([128, 4, IPC], F32, tag="flex")
            nc.vector.tensor_copy(f1T_ps, b1pT[:, :, None].to_broadcast(
                [128, 4, IPC]))
            for db in range(4):
                for hb in range(4):
                    nc.tensor.matmul(f1T_ps[:, hb, :],
                                     W1T_sb[:, db, 128 * hb:128 * hb + 128],
                                     o1T_sb[:, db, :],
                                     start=False, stop=(db == 3),
                                     skip_group_check=True)
            f1T_sb = postp.tile([128, 4, IPC], F16)
            nc.vector.tensor_scalar_max(f1T_sb, f1T_ps, 0.0)

            f2_ps = psu.tile([IPC, H], F32, tag="flex")
            for hb in range(4):
                nc.tensor.matmul(f2_ps, f1T_sb[:, hb, :], W2T_sb[:, hb, :],
                                 start=(hb == 0), stop=(hb == 3))

            # ---- residual + LN2 (fused epilogue) ----
            h2 = postp.tile([IPC, H], F32)
            nc.vector.tensor_add(h2, f2_ps, o1b)
            mv2, rstd2 = ln_stats(h2)
            t2 = postp.tile([IPC, H], F32, tag="t2")
            nc.vector.scalar_tensor_tensor(
                t2, h2, mv2[:, 0:1], gb_sb, OP.subtract, OP.mult)
            out2 = postp.tile([IPC, H], F32, tag="out2")
            nc.vector.scalar_tensor_tensor(
                out2, t2, rstd2, bb_sb, OP.mult, OP.add)

            nc.sync.dma_start(out=out_d, in_=out2)
            if DEBUG:
                nc.sync.dma_start(out=dbg_alpha, in_=alpha_full)
                nc.sync.dma_start(out=dbg_h, in_=h_sb)
                nc.sync.dma_start(out=dbg_cen, in_=cen)
                nc.sync.dma_start(out=dbg_f1, in_=f1T_sb)
                nc.sync.dma_start(out=dbg_gxT, in_=gxT_sb)
                nc.sync.dma_start(out=dbg_u, in_=u_sb)

    return nc


def prep_in_maps(inputs) -> list[dict]:
    x = np.asarray(inputs["x"], np.float32)
    pf = np.asarray(inputs["pair_feats"], np.float32)
    W_att = np.asarray(inputs["W_att"], np.float32)
    b_att = np.asarray(inputs["b_att"], np.float32)
    W_obj = np.asarray(inputs["W_obj"], np.float32)
    b_obj = np.asarray(inputs["b_obj"], np.float32)
    W_pair = np.asarray(inputs["W_pair"], np.float32)
    b_pair = np.asarray(inputs["b_pair"], np.float32)
    ln_g = np.asarray(inputs["ln_g"], np.float32)
    ln_b = np.asarray(inputs["ln_b"], np.float32)
    W1 = np.asarray(inputs["W1"], np.float32)
    b1 = np.asarray(inputs["b1"], np.float32)
    W2 = np.asarray(inputs["W2"], np.float32)
    b2 = np.asarray(inputs["b2"], np.float32)

    wa, wb, wc = W_att[0, :D], W_att[0, D:2 * D], W_att[0, 2 * D:]
    xpad = np.concatenate([x, np.zeros((1, D), np.float32)], axis=0)

    # fold wc into pf columns; recover U via pre-divided W_pair.T rows.
    colscale = np.sign(wc) * np.maximum(np.abs(wc), 6e-5)
    colscale[colscale == 0] = 6e-5
    # 1/511 (the mean over neighbors) is folded into the three weight paths
    # that consume raw alpha: U@WpT, (A@x)@WoT, and s_alpha*bop.
    WpT2 = (W_pair.T / colscale[:, None] / T).astype(np.float16)
    WoT2 = (W_obj.T / T).astype(np.float16)
    dxf = np.diff(xpad[:K + 1], axis=0)
    b1p = b1 + ln_b @ W1.T

    # b32: [sh1 0:128 | sh2 128:256 | poison 256 | b_att 257 | b1pT 258:262]
    b32a = np.zeros((128, 262), np.float32)
    b32a[:, 0:128] = np.eye(128, k=-1)      # shift1[q, p] = (q == p+1)
    b32a[0, 128 + 127] = 1.0                 # shift2[q, p] = (q==0)&(p==127)
    b32a[127, 256] = -1e9                    # poison col (t=511 pad slot)
    b32a[:, 257] = b_att[0]
    b32a[:, 258:262] = b1p.reshape(4, 128).T
    # b16: [wb 0:512 | mge 512:768 | ones 768:896 | wa4 896:900
    #       | xiT 900:1156 | sel 1156:1172 | id16 1172:1300]
    b16a = np.zeros((128, 1300), np.float16)
    b16a[:, 0:512] = wb[None, :]
    b16a[:, 768:896] = 1.0
    b16a[:, 896:900] = wa.reshape(4, 128).T
    q = np.arange(128)
    for j in range(4):
        for s in range(4):
            b16a[:, 1156 + 4 * j + s] = (q == 32 * s + j)
    b16a[:, 1172:1300] = np.eye(128, dtype=np.float16)
    xlo_np = np.ascontiguousarray(
        x.reshape(NCH, 128, D).transpose(1, 0, 2)).astype(np.float16)

    base = dict(
        xlo_ch=xlo_np,
        dxf=dxf.astype(np.float16),
        bias4=np.stack([ln_g, ln_b, ln_b + b2,
                        (b_obj + b_pair) / T]).astype(np.float32),
        WpT=np.ascontiguousarray(WpT2),
        WoT=np.ascontiguousarray(WoT2),
        W1T=np.ascontiguousarray(W1.T * ln_g[:, None]).astype(np.float16),
        W2T=np.ascontiguousarray(W2.T).astype(np.float16),
    )

    pfr = pf.reshape(K, T, PD)
    tgrid = np.arange(128)[:, None] + 128 * np.arange(NCH)[None, :]   # [128, NCH]

    in_maps = []
    for core in range(NCORES):
        ig = np.arange(core * IPC, (core + 1) * IPC)
        mge = ((tgrid[:, :, None] >= ig[None, None, :])
               & (tgrid[:, :, None] <= T - 1)).astype(np.float16)
        # [chunk, t, i, pd] layout -> each tile DMA is one contiguous burst
        shard = np.zeros((NCH * 128, IPC, PD), np.float16)
        shard[:T] = (pfr[ig] * colscale[None, None, :]).transpose(1, 0, 2)
        # split into i-halves: [chunk, half, t, 32, pd]
        pf_shard = np.ascontiguousarray(
            shard.reshape(NCH, 128, 2, 32, PD).transpose(0, 2, 1, 3, 4))
        xi = x[ig]
        cb32 = b32a
        cb16 = b16a.copy()
        cb16[:, 512:768] = mge.reshape(128, NCH * IPC)
        cb16[:, 900:1156] = xi.T.reshape(4, 128, IPC).transpose(
            1, 0, 2).reshape(128, 4 * IPC).astype(np.float16)
        m = dict(base)
        m.update(
            pf=pf_shard,
            xi=xi.astype(np.float32),
            b32=cb32,
            b16=cb16,
        )
        in_maps.append(m)
    return in_maps


_COMPILED = None


def _get_program() -> bacc.Bacc:
    global _COMPILED
    if _COMPILED is None:
        nc = build_program()
        nc.compile()
        _COMPILED = nc
    return _COMPILED


TRACE = False
LAST_RESULT = None


def _install_axon_ntff_hook():
    """The container's antenv lacks axon_hooks; recreate it from trn_boot's
    ctypes implementation so trace=True can capture NTFF profiles."""
    import sys
    import types
    try:
        from antenv.axon_hooks import get_axon_ntff_profile_hook  # noqa: F401
        return
    except ImportError:
        pass
    from trn_agent_boot.trn_boot import _ntff_profile_via_ctypes
    hook = _ntff_profile_via_ctypes("/opt/axon/libaxon_pjrt.so")
    m = types.ModuleType("antenv.axon_hooks")
    m.get_axon_ntff_profile_hook = lambda: hook
    sys.modules["antenv.axon_hooks"] = m


def kernel(**inputs) -> np.ndarray:
    import concourse.bass_utils as bu
    from concourse.bass_utils import run_bass_kernel_spmd
    global LAST_RESULT
    if TRACE:
        _install_axon_ntff_hook()
        bu.upload_artifacts = lambda tmpdir: str(tmpdir)  # no bucket here
    nc = _get_program()
    in_maps = prep_in_maps(inputs)
    res = run_bass_kernel_spmd(nc, in_maps, list(range(NCORES)), trace=TRACE)
    LAST_RESULT = res
    outs = [res.results[c]["out"] for c in range(NCORES)]
    return np.concatenate(outs, axis=0).astype(np.float32)


# revision 14
# speedup vs baseline: 1.1067x; 1.1067x over previous
"""Trainium2 Bass kernel for nn_ARTLayer (gnn_message_passing).

Math (reference):
    j(i,t) = t + (t>=i)                                    # [K, K-1] neighbor index
    alpha  = sigmoid(x@wa [i] + x@wb [j] + pf@wc + b_att)  # [K, K-1]
    msgs   = mean_t alpha * ((x@WobjT + b_obj)[j] + pf@WpairT + b_pair)
    out    = LN(x + msgs); out = LN(out + FFN(out))

Key algebraic rewrite (removes the 34-GFLOP [P,PD]x[PD,H] einsum):
    sum_t a*(pf@WpT)  = (sum_t a*pf) @ WpT               -> U[i,:] @ WpT
    sum_t a*oj[j]     = (A @ x) @ WobjT                  -> Gx[i,:] @ WoT
      with A[i,j] decomposed via lo/hi shifted views of x and a t>=i mask
    sum_t a*(b_obj+b_pair) = s_alpha[i] * bop

Sharding: rows i split across 8 cores (64 each); small tensors replicated;
host concatenates the per-core [64, 512] outputs.

V2 schedule (rebuilt from NTFF trace analysis of the 88us baseline):
  - pf streams on the sync HWDGE queue right behind the packed consts, in
    8 half-chunk DMAs, so the DVE reduce tree starts at ~6us instead of 29us.
  - sa/sb use fused scalar_tensor_tensor with accum_out (one op instead of
    mul+reduce); tree tail levels fused into one tensor_reduce.
  - Gx and the dx correction accumulate *transposed* in PSUM (xlo/dx blocks
    as the matmul stationary, alpha as the moving operand) - no PE
    transposes of gx in the tail. s_alpha accumulated as a column the same
    way.
  - U diagonal extraction streams a [128,4] selection matrix instead of a
    full 128-col transpose; one 4D-AP gather copy assembles U^T.
  - msg/f1 biases are preloaded into PSUM by the DVE (no rank-1 matmuls in
    PE accumulation groups); FFN layer 1 computed directly transposed
    (W1T blocks stationary) so only one transpose set (out1) remains.
  - LN epilogues fused via scalar_tensor_tensor; sqrt table switch forced
    right after the last sigmoid via a dummy sqrt.
  - Tail-only weights ride the ACT HWDGE queue, position-staggered between
    sigmoids so they never contend with the pf stream for HBM.
"""
import numpy as np

import concourse.bass as bass
import concourse.tile as tile
from concourse import bacc, mybir

F32, F16 = mybir.dt.float32, mybir.dt.float16
AX = mybir.AxisListType
OP = mybir.AluOpType
AF = mybir.ActivationFunctionType

DEBUG = False
K, D, H, PD = 512, 512, 512, 128
T = K - 1                      # 511 neighbors per row
NCORES, IPC = 8, 64            # rows per core
NCH = 4                        # t-chunks of 128 (last chunk row 127 is t=511 pad)
IB = 64                        # i-block within a core
EPS = 1e-5


def build_program() -> bacc.Bacc:
    nc = bacc.Bacc("TRN2", target_bir_lowering=False, debug=False)

    def inp(name, shape, dt):
        return nc.dram_tensor(name, shape, dt, kind="ExternalInput").ap()

    pf = inp("pf", [NCH, 2, 128, 32, PD], F16)  # [chunk, i-half, t, i, pd]
    # single-DMA packed critical constants (128 contiguous rows each):
    # b32: [cmat 0:512 | poison 512:640 | mask_lt 640:896 | b_att 896:904
    #       | b1pT 904:908]
    # b16: [wa_t 0:512 | wb_t 512:1024 | mask_ge 1024:1280 | ones 1280:1288
    #       | xi16 1288:1800 | sel 1800:1816 | id16 1816:1944]
    b32 = inp("b32", [128, 262], F32)
    b16 = inp("b16", [128, 1300], F16)
    xlo_ch = inp("xlo_ch", [128, NCH, D], F16)  # x rows chunked [t%128, t//128]
    dxf = inp("dxf", [K, D], F16)               # x[t+1] - x[t], host computed
    xi = inp("xi", [IPC, D], F32)               # this core's rows of x
    bias4 = inp("bias4", [4, H], F32)           # [ln_g; ln_b; lnb+b2; bop/511]
    WpT = inp("WpT", [PD, H], F16)              # W_pair.T / colscale / 511
    WoT = inp("WoT", [D, H], F16)               # W_obj.T / 511
    W1T = inp("W1T", [H, H], F16)               # W1.T * ln_g
    W2T = inp("W2T", [H, H], F16)

    out_d = nc.dram_tensor("out", [IPC, H], F32, kind="ExternalOutput").ap()
    if DEBUG:
        dbg_alpha = nc.dram_tensor(
            "dbg_alpha", [128, NCH, IPC], F16, kind="ExternalOutput").ap()
        dbg_h = nc.dram_tensor("dbg_h", [IPC, H], F32, kind="ExternalOutput").ap()
        dbg_cen = nc.dram_tensor("dbg_cen", [IPC, H], F16, kind="ExternalOutput").ap()
        dbg_f1 = nc.dram_tensor(
            "dbg_f1", [128, 4, IPC], F16, kind="ExternalOutput").ap()
        dbg_gxT = nc.dram_tensor(
            "dbg_gxT", [128, 4, IPC], F16, kind="ExternalOutput").ap()
        dbg_u = nc.dram_tensor("dbg_u", [128, IPC], F16, kind="ExternalOutput").ap()

    with tile.TileContext(nc) as tc:
        with (
            tc.tile_pool(name="const", bufs=1) as cpool,
            tc.tile_pool(name="pfp", bufs=4) as pfp,
            tc.tile_pool(name="scrp", bufs=2) as scrp,
            tc.tile_pool(name="smallp", bufs=4) as smallp,
            tc.tile_pool(name="postp", bufs=3) as postp,
            tc.tile_pool(name="pss", bufs=2, space="PSUM") as pss,
            tc.tile_pool(name="psu", bufs=4, space="PSUM") as psu,
            tc.tile_pool(name="psg", bufs=1, space="PSUM") as psg,
            tc.tile_pool(name="psm", bufs=1, space="PSUM") as psm,
        ):
            def dma(out, in_):
                nc.sync.dma_start(out=out, in_=in_)

            def dma_w(out, in_):
                nc.scalar.dma_start(out=out, in_=in_)

            # ---- critical consts, then the pf stream, on the sync queue ----
            hp = tc.high_priority()
            hp.__enter__()
            b16_sb = cpool.tile([128, 1300], F16)
            dma(b16_sb, b16)
            xlo = cpool.tile([128, NCH, D], F16)
            dma(xlo, xlo_ch)
            b32_sb = cpool.tile([128, 262], F32)
            dma(b32_sb, b32)
            hp.__exit__(None, None, None)
            tc.no_sync_barrier()   # keep pf DMAs behind the critical consts

            pf_t = []
            for c in range(NCH):
                t_ = pfp.tile([128, IB, PD], F16, tag="pf_t", name=f"pf{c}")
                pf_t.append(t_)
                for h in range(2):
                    dma(t_[:, 32 * h:32 * h + 32, :], pf[c, h])

            sh1_sb = b32_sb[:, 0:128]
            sh2_sb = b32_sb[:, 128:256]
            poison_col = b32_sb[:, 256:257]
            b_att_1 = b32_sb[0:1, 257:258]
            b1pT = b32_sb[:, 258:262]
            wb_b = b16_sb[:, 0:512]
            mge_sb = b16_sb[:, 512:768].rearrange("p (c i) -> p c i", c=NCH)
            ones16_col = b16_sb[:, 768:769]
            ones16_row = b16_sb[0:1, 768:896]
            wa4 = b16_sb[:, 896:900]
            xiT_sb = b16_sb[:, 900:1156].rearrange("p (c i) -> p c i", c=NCH)
            sel_sb = b16_sb[:, 1156:1172].rearrange("p (j s) -> p j s", j=4)
            id16_sb = b16_sb[:, 1172:1300]

            eps_col = cpool.tile([IPC, 1], F32)
            nc.vector.memset(eps_col, EPS)
            zero_col = cpool.tile([128, 1], F32)
            nc.vector.memset(zero_col, 0.0)

            # ---- sb (all rows) fused; sa broadcast via a rank-1 matmul ----
            # sbj[t,i] = sa[i] + b_att + sb_lo[t] + mge[t,i]*(sb_hi-sb_lo)[t]
            # (poison -1e9 folded into sb_lo at the t=511 pad slot)
            sb_cols = smallp.tile([128, NCH], F32)
            for c in range(NCH):
                scr_sb = smallp.tile([128, D], F16, tag="scr_sb")
                nc.vector.scalar_tensor_tensor(
                    scr_sb, xlo[:, c, :], 0.0, wb_b, OP.bypass, OP.mult,
                    accum_out=sb_cols[:, c:c + 1])

            # sb_hi[p, c] = sb[c*128+p+1] via shift matmuls; slot 511 stays 0
            sbhi_ps = pss.tile([128, NCH], F32, tag="ps_small")
            nc.tensor.matmul(sbhi_ps, sh1_sb, sb_cols, start=True, stop=False)
            nc.tensor.matmul(sbhi_ps[:, 0:NCH - 1], sh2_sb, sb_cols[:, 1:NCH],
                             start=False, stop=True)
            dneg = smallp.tile([128, NCH], F32)
            nc.vector.tensor_tensor(dneg, sbhi_ps, sb_cols, OP.subtract)
            nc.vector.tensor_add(sb_cols[:, 3:4], sb_cols[:, 3:4], poison_col)

            # sa row: wa-block columns stationary over xT blocks, then a
            # rank-1 ones matmul broadcasts it across all t partitions
            sarow_ps = pss.tile([1, IPC], F32, tag="ps_small")
            for db in range(4):
                nc.tensor.matmul(sarow_ps, wa4[:, db:db + 1],
                                 xiT_sb[:, db, :],
                                 start=(db == 0), stop=(db == 3))
            sarow16 = smallp.tile([1, IPC], F16)
            nc.vector.tensor_scalar(sarow16, sarow_ps, b_att_1, None,
                                    OP.add, OP.bypass)
            x_ps = pss.tile([128, IPC], F32, tag="ps_small")
            nc.tensor.matmul(x_ps, ones16_row, sarow16, start=True, stop=True)

            sbj = cpool.tile([128, NCH, IPC], F32)
            for c in range(NCH):
                x3c = smallp.tile([128, IPC], F32, tag="x3c")
                nc.vector.tensor_scalar(x3c, x_ps, sb_cols[:, c:c + 1], None,
                                        OP.add, OP.bypass)
                nc.vector.scalar_tensor_tensor(
                    sbj[:, c, :], mge_sb[:, c, :], dneg[:, c:c + 1], x3c,
                    OP.mult, OP.add)

            # ---- main edge pass ----
            alpha_full = cpool.tile([128, NCH, IPC], F16)   # raw sigmoid out
            age_full = cpool.tile([128, NCH, IPC], F16)     # masked (t>=i) alpha
            # gxT[d, i] (4 d-blocks) + s_alpha column, one PSUM bank
            gs_ps = psg.tile([128, 5, IPC], F32)
            nc.vector.memset(gs_ps, 0.0)
            u_ps = [psu.tile([128, 512], F32, tag="flex", name=f"u_ps{b}")
                    for b in range(4)]
            for b in range(4):
                nc.vector.memset(u_ps[b], 0.0)

            def late_dma(dst, src_dram, when):
                # fake WAW: a 1-elem DVE copy into dst that reads `when`
                # pins the weight DMA behind that chunk's sigmoid, so the
                # transfer never races the pf stream for HBM.
                nc.vector.tensor_copy(dst[0:1, 0:1], when)
                dma_w(dst, src_dram)

            for c in range(NCH):
                scr = scrp.tile([128, IB, 64], F16, tag="scr")
                # halved L1/L2 so the tree starts as soon as each half lands
                for h in range(2):
                    ih = slice(32 * h, 32 * h + 32)
                    nc.vector.tensor_add(
                        scr[:, ih, :], pf_t[c][:, ih, 0:64],
                        pf_t[c][:, ih, 64:128])
                    nc.vector.tensor_add(
                        scr[:, ih, 0:32], scr[:, ih, 0:32], scr[:, ih, 32:64])
                nc.vector.tensor_add(
                    scr[:, :, 0:16], scr[:, :, 0:16], scr[:, :, 16:32])
                nc.vector.tensor_add(
                    scr[:, :, 0:8], scr[:, :, 0:8], scr[:, :, 8:16])
                sc_t = smallp.tile([128, IB], F32, tag="sc_t")
                nc.vector.tensor_reduce(sc_t, scr[:, :, 0:8], axis=AX.X,
                                        op=OP.add)
                aarg = smallp.tile([128, IB], F32, tag="aarg")
                nc.vector.tensor_add(aarg, sc_t, sbj[:, c, :])
                nc.scalar.activation(alpha_full[:, c, :], aarg, AF.Sigmoid,
                                     bias=zero_col)
                nc.vector.tensor_mul(age_full[:, c, :], alpha_full[:, c, :],
                                     mge_sb[:, c, :])
                if c == 0:
                    dx = cpool.tile([128, NCH, D], F16)
                    late_dma(dx.rearrange("p c d -> p (c d)"),
                             dxf.rearrange("(c p) d -> p c d", p=128),
                             alpha_full[0:1, 0, 0:1])
                elif c == 1:
                    WoT_sb = cpool.tile([128, NCH, H], F16)
                    late_dma(WoT_sb.rearrange("p c h -> p (c h)"),
                             WoT.rearrange("(c p) h -> p c h", p=128),
                             alpha_full[0:1, 1, 0:1])
                    WpT_sb = cpool.tile([128, H], F16)
                    late_dma(WpT_sb, WpT, alpha_full[0:1, 1, 0:1])
                elif c == 2:
                    xi_sb = cpool.tile([IPC, D], F32)
                    late_dma(xi_sb, xi, alpha_full[0:1, 2, 0:1])
                    bias_sb = cpool.tile([IPC, 4, H], F32)
                    late_dma(bias_sb.rearrange("p c h -> p (c h)"),
                             bias4[None, :, :].to_broadcast([IPC, 4, H]),
                             alpha_full[0:1, 2, 0:1])
                # U quads: lhsT = 4 alpha columns, rhs = 4 pf blocks; the
                # wanted rows sit on the block diagonal
                for q in range(IB // 4):
                    b, sp = divmod(q, 4)
                    nc.tensor.matmul(
                        u_ps[b][32 * sp:32 * sp + 4, :],
                        alpha_full[:, c, 4 * q:4 * q + 4],
                        pf_t[c][:, 4 * q:4 * q + 4, :],
                        start=(c == 0), stop=(c == NCH - 1),
                        tile_position=(0, 32 * sp))
                # gxT accumulates transposed: xlo block stationary, alpha
                # moving.  s_alpha as a column via the ones vector.
                for db in range(4):
                    nc.tensor.matmul(gs_ps[:, db, :],
                                     xlo[:, c, 128 * db:128 * db + 128],
                                     alpha_full[:, c, :],
                                     start=False, stop=False,
                                     skip_group_check=True)
                nc.tensor.matmul(gs_ps[0:IPC, 4, 0:1], alpha_full[:, c, :],
                                 ones16_col, start=False,
                                 stop=(c == NCH - 1), skip_group_check=True)
                # dx correction for the PREVIOUS chunk (dx lands mid-loop)
                if c >= 1:
                    for db in range(4):
                        nc.tensor.matmul(gs_ps[:, db, :],
                                         dx[:, c - 1, 128 * db:128 * db + 128],
                                         age_full[:, c - 1, :],
                                         start=False, stop=False,
                                         skip_group_check=True)
                tc.no_sync_barrier()

            gb_sb = bias_sb[:, 0, :]
            bb_sb = bias_sb[:, 1, :]
            lb2_sb = bias_sb[:, 2, :]
            bop_sb = bias_sb[:, 3, :]

            # scheduler fence: keep every tail instruction after the loop in
            # each engine stream (strict-FIFO engines head-of-line block)
            tc.no_sync_barrier()

            for db in range(4):
                nc.tensor.matmul(gs_ps[:, db, :],
                                 dx[:, 3, 128 * db:128 * db + 128],
                                 age_full[:, 3, :],
                                 start=False, stop=True,
                                 skip_group_check=True)

            # ---- messages = U@WpT + GxT^T@WoT + s_alpha x bop ----
            msg_ps = psm.tile([IPC, H], F32)

            # bop rank-1 preloaded into PSUM by the DVE (no PE rank-1 matmul)
            s_col = smallp.tile([IPC, 1], F32)
            nc.vector.tensor_copy(s_col, gs_ps[0:IPC, 4, 0:1])
            gxT_sb = postp.tile([128, 4, IPC], F16)
            nc.vector.tensor_copy(gxT_sb, gs_ps[:, 0:4, :])
            nc.vector.tensor_scalar(msg_ps, bop_sb, s_col, None,
                                    OP.mult, OP.bypass)

            for db in range(4):
                nc.tensor.matmul(msg_ps, gxT_sb[:, db, :], WoT_sb[:, db, :],
                                 start=False, stop=False, skip_group_check=True)

            # U extraction: copy each PSUM bank to SBUF (2 on DVE, 2 on ACT),
            # then a [128,4] selection matmul per (bank, col-block) pulls the
            # diagonal quads out transposed; one 4D gather copy assembles U^T.
            slots_ps = pss.tile([128, IB], F32, tag="ps_small")
            slots3 = slots_ps.rearrange("p (g s) -> p g s", g=16)
            ucp = []
            for b in range(4):
                u_cp = postp.tile([128, 512], F16, tag="u_cp", name=f"ucp{b}")
                ucp.append(u_cp)
                if b % 2 == 0:
                    nc.vector.tensor_copy(u_cp, u_ps[b])
                else:
                    nc.scalar.copy(u_cp, u_ps[b])
            for b in range(4):
                for j in range(4):
                    nc.tensor.matmul(slots3[:, 4 * b + j, :],
                                     ucp[b][:, 128 * j:128 * j + 128],
                                     sel_sb[:, j, :],
                                     start=True, stop=True,
                                     skip_group_check=True)
            # sqrt table switch + late FFN weights ride the ACT queue
            # behind the ucp copies (anchored so they cannot hoist)
            junk = smallp.tile([1, 1], F32, tag="junk")
            nc.scalar.activation(junk, eps_col[0:1, :], AF.Sqrt)
            W1T_sb = cpool.tile([128, NCH, H], F16)
            late_dma(W1T_sb.rearrange("p c h -> p (c h)"),
                     W1T.rearrange("(c p) h -> p c h", p=128),
                     ucp[1][0:1, 0:1])
            W2T_sb = cpool.tile([128, NCH, H], F16)
            late_dma(W2T_sb.rearrange("p c h -> p (c h)"),
                     W2T.rearrange("(c p) h -> p c h", p=128),
                     ucp[1][0:1, 0:1])

            u_sb = postp.tile([128, IB], F16)
            nc.vector.tensor_copy(
                u_sb.rearrange("p (b s j) -> p b s j", b=4, s=4),
                slots_ps.rearrange("p (b j s) -> p b s j", b=4, j=4))
            nc.tensor.matmul(msg_ps, u_sb, WpT_sb, start=False, stop=True,
                             skip_group_check=True)

            # ---- residual + LN1 (bare: ln_g/ln_b folded into W1T'/b1p) ----
            def ln_stats(v):
                stats = smallp.tile([IPC, 6], F32, tag="stats")
                nc.vector.bn_stats(out=stats, in_=v)
                mv = smallp.tile([IPC, 2], F32, tag="mv")
                nc.vector.bn_aggr(out=mv, in_=stats)
                std = smallp.tile([IPC, 1], F32, tag="std")
                nc.scalar.activation(std, mv[:, 1:2], AF.Sqrt, bias=eps_col)
                rstd = smallp.tile([IPC, 1], F32, tag="rstd")
                nc.vector.reciprocal(rstd, std)
                return mv, rstd

            h_sb = postp.tile([IPC, H], F32)
            nc.vector.tensor_add(h_sb, xi_sb, msg_ps)
            mv1, rstd1 = ln_stats(h_sb)
            cen = postp.tile([IPC, H], F16)
            nc.vector.tensor_scalar(cen, h_sb, mv1[:, 0:1], rstd1,
                                    OP.subtract, OP.mult)

            # ---- FFN: f1T computed directly transposed (W1T stationary) ----
            o1T_ps = pss.tile([128, 4, IPC], F16, tag="ps_small")
            for db in range(4):
                nc.tensor.transpose(o1T_ps[:, db, :],
                                    cen[:, 128 * db:128 * db + 128],
                                    id16_sb[0:IPC, 0:IPC])
            o1T_sb = postp.tile([128, 4, IPC], F16)
            nc.vector.tensor_copy(o1T_sb, o1T_ps)

            # true out1*g + (ln_b + b2) rebuilt off the critical path
            o1b = postp.tile([IPC, H], F32)
            nc.vector.scalar_tensor_tensor(
                o1b, cen, 0.0, gb_sb, OP.bypass, OP.mult)
            nc.vector.tensor_add(o1b, o1b, lb2_sb)

            f1T_ps = psu.tile([128, 4, IPC], F32, tag="flex")
            nc.vector.tensor_copy(f1T_ps, b1pT[:, :, None].to_broadcast(
                [128, 4, IPC]))
            for db in range(4):
                for hb in range(4):
                    nc.tensor.matmul(f1T_ps[:, hb, :],
                                     W1T_sb[:, db, 128 * hb:128 * hb + 128],
                                     o1T_sb[:, db, :],
                                     start=False, stop=(db == 3),
                                     skip_group_check=True)
            f1T_sb = postp.tile([128, 4, IPC], F16)
            nc.vector.tensor_scalar_max(f1T_sb, f1T_ps, 0.0)

            f2_ps = psu.tile([IPC, H], F32, tag="flex")
            for hb in range(4):
                nc.tensor.matmul(f2_ps, f1T_sb[:, hb, :], W2T_sb[:, hb, :],
                                 start=(hb == 0), stop=(hb == 3))

            # ---- residual + LN2 (fused epilogue) ----
            h2 = postp.tile([IPC, H], F32)
            nc.vector.tensor_add(h2, f2_ps, o1b)
            mv2, rstd2 = ln_stats(h2)
            t2 = postp.tile([IPC, H], F32, tag="t2")
            nc.vector.scalar_tensor_tensor(
                t2, h2, mv2[:, 0:1], gb_sb, OP.subtract, OP.mult)
            out2 = postp.tile([IPC, H], F32, tag="out2")
            nc.vector.scalar_tensor_tensor(
                out2, t2, rstd2, bb_sb, OP.mult, OP.add)

            nc.sync.dma_start(out=out_d, in_=out2)
            if DEBUG:
                nc.sync.dma_start(out=dbg_alpha, in_=alpha_full)
                nc.sync.dma_start(out=dbg_h, in_=h_sb)
                nc.sync.dma_start(out=dbg_cen, in_=cen)
                nc.sync.dma_start(out=dbg_f1, in_=f1T_sb)
                nc.sync.dma_start(out=dbg_gxT, in_=gxT_sb)
                nc.sync.dma_start(out=dbg_u, in_=u_sb)

    return nc


def prep_in_maps(inputs) -> list[dict]:
    x = np.asarray(inputs["x"], np.float32)
    pf = np.asarray(inputs["pair_feats"], np.float32)
    W_att = np.asarray(inputs["W_att"], np.float32)
    b_att = np.asarray(inputs["b_att"], np.float32)
    W_obj = np.asarray(inputs["W_obj"], np.float32)
    b_obj = np.asarray(inputs["b_obj"], np.float32)
    W_pair = np.asarray(inputs["W_pair"], np.float32)
    b_pair = np.asarray(inputs["b_pair"], np.float32)
    ln_g = np.asarray(inputs["ln_g"], np.float32)
    ln_b = np.asarray(inputs["ln_b"], np.float32)
    W1 = np.asarray(inputs["W1"], np.float32)
    b1 = np.asarray(inputs["b1"], np.float32)
    W2 = np.asarray(inputs["W2"], np.float32)
    b2 = np.asarray(inputs["b2"], np.float32)

    wa, wb, wc = W_att[0, :D], W_att[0, D:2 * D], W_att[0, 2 * D:]
    xpad = np.concatenate([x, np.zeros((1, D), np.float32)], axis=0)

    # fold wc into pf columns; recover U via pre-divided W_pair.T rows.
    colscale = np.sign(wc) * np.maximum(np.abs(wc), 6e-5)
    colscale[colscale == 0] = 6e-5
    # 1/511 (the mean over neighbors) is folded into the three weight paths
    # that consume raw alpha: U@WpT, (A@x)@WoT, and s_alpha*bop.
    WpT2 = (W_pair.T / colscale[:, None] / T).astype(np.float16)
    WoT2 = (W_obj.T / T).astype(np.float16)
    dxf = np.diff(xpad[:K + 1], axis=0)
    b1p = b1 + ln_b @ W1.T

    # b32: [sh1 0:128 | sh2 128:256 | poison 256 | b_att 257 | b1pT 258:262]
    b32a = np.zeros((128, 262), np.float32)
    b32a[:, 0:128] = np.eye(128, k=-1)      # shift1[q, p] = (q == p+1)
    b32a[0, 128 + 127] = 1.0                 # shift2[q, p] = (q==0)&(p==127)
    b32a[127, 256] = -1e9                    # poison col (t=511 pad slot)
    b32a[:, 257] = b_att[0]
    b32a[:, 258:262] = b1p.reshape(4, 128).T
    # b16: [wb 0:512 | mge 512:768 | ones 768:896 | wa4 896:900
    #       | xiT 900:1156 | sel 1156:1172 | id16 1172:1300]
    b16a = np.zeros((128, 1300), np.float16)
    b16a[:, 0:512] = wb[None, :]
    b16a[:, 768:896] = 1.0
    b16a[:, 896:900] = wa.reshape(4, 128).T
    q = np.arange(128)
    for j in range(4):
        for s in range(4):
            b16a[:, 1156 + 4 * j + s] = (q == 32 * s + j)
    b16a[:, 1172:1300] = np.eye(128, dtype=np.float16)
    xlo_np = np.ascontiguousarray(
        x.reshape(NCH, 128, D).transpose(1, 0, 2)).astype(np.float16)

    base = dict(
        xlo_ch=xlo_np,
        dxf=dxf.astype(np.float16),
        bias4=np.stack([ln_g, ln_b, ln_b + b2,
                        (b_obj + b_pair) / T]).astype(np.float32),
        WpT=np.ascontiguousarray(WpT2),
        WoT=np.ascontiguousarray(WoT2),
        W1T=np.ascontiguousarray(W1.T * ln_g[:, None]).astype(np.float16),
        W2T=np.ascontiguousarray(W2.T).astype(np.float16),
    )

    pfr = pf.reshape(K, T, PD)
    tgrid = np.arange(128)[:, None] + 128 * np.arange(NCH)[None, :]   # [128, NCH]

    in_maps = []
    for core in range(NCORES):
        ig = np.arange(core * IPC, (core + 1) * IPC)
        mge = ((tgrid[:, :, None] >= ig[None, None, :])
               & (tgrid[:, :, None] <= T - 1)).astype(np.float16)
        # [chunk, t, i, pd] layout -> each tile DMA is one contiguous burst
        shard = np.zeros((NCH * 128, IPC, PD), np.float16)
        shard[:T] = (pfr[ig] * colscale[None, None, :]).transpose(1, 0, 2)
        # split into i-halves: [chunk, half, t, 32, pd]
        pf_shard = np.ascontiguousarray(
            shard.reshape(NCH, 128, 2, 32, PD).transpose(0, 2, 1, 3, 4))
        xi = x[ig]
        cb32 = b32a
        cb16 = b16a.copy()
        cb16[:, 512:768] = mge.reshape(128, NCH * IPC)
        cb16[:, 900:1156] = xi.T.reshape(4, 128, IPC).transpose(
            1, 0, 2).reshape(128, 4 * IPC).astype(np.float16)
        m = dict(base)
        m.update(
            pf=pf_shard,
            xi=xi.astype(np.float32),
            b32=cb32,
            b16=cb16,
        )
        in_maps.append(m)
    return in_maps


_COMPILED = None


def _get_program() -> bacc.Bacc:
    global _COMPILED
    if _COMPILED is None:
        nc = build_program()
        nc.compile()
        _COMPILED = nc
    return _COMPILED


TRACE = False
LAST_RESULT = None


def _install_axon_ntff_hook():
    """The container's antenv lacks axon_hooks; recreate it from trn_boot's
    ctypes implementation so trace=True can capture NTFF profiles."""
    import sys
    import types
    try:
        from antenv.axon_hooks import get_axon_ntff_profile_hook  # noqa: F401
        return
    except ImportError:
        pass
    from trn_agent_boot.trn_boot import _ntff_profile_via_ctypes
    hook = _ntff_profile_via_ctypes("/opt/axon/libaxon_pjrt.so")
    m = types.ModuleType("antenv.axon_hooks")
    m.get_axon_ntff_profile_hook = lambda: hook
    sys.modules["antenv.axon_hooks"] = m


def kernel(**inputs) -> np.ndarray:
    import concourse.bass_utils as bu
    from concourse.bass_utils import run_bass_kernel_spmd
    global LAST_RESULT
    if TRACE:
        _install_axon_ntff_hook()
        bu.upload_artifacts = lambda tmpdir: str(tmpdir)  # no bucket here
    nc = _get_program()
    in_maps = prep_in_maps(inputs)
    res = run_bass_kernel_spmd(nc, in_maps, list(range(NCORES)), trace=TRACE)
    LAST_RESULT = res
    outs = [res.results[c]["out"] for c in range(NCORES)]
    return np.concatenate(outs, axis=0).astype(np.float32)
